# revision 1
# baseline (speedup 1.0000x reference)
"""Multi-head attention (B=2, N=2048, E=1024, H=16) on 8 Trainium2 NeuronCores.

Sharding: data-parallel over batch (2) x tensor-parallel over head-groups (4
groups of 4 heads).  Core c handles batch c//4 and heads 4*(c%4)..4*(c%4)+3.

Host-side shard prep feeds each core feature-major (transposed) activations
and weight shards; the device kernel computes
  qT = Wq_s @ xT + bq_s        (feature-major, [256, 2048])
  kT = Wk_s @ xT + bk_s
  v  = x @ Wv_s.T + bv_s       (position-major, [2048, 256], bf16)
  eT[kpos, q] per head          (transposed energy, head pairs row-packed
                                 into the PE array, K=64 each)
  s = exp(eT)  (bf16)           (no max-subtraction: |logits| < ~60 << 88)
  o  = s.T @ [v | 32]           (32-column yields 32*rowsum in psum row 64)
  oT normalized by 1/(32*rowsum)   (= softmax / sqrt(E) module quirk)
  out_partial = oT.T @ Wp[:, cols].T   (position-major [2048, 1024])
Host sums the 4 head-group partials per batch and adds bp.

Matmuls run in float32r (full-rate fp32 PE mode, ~2e-4 relative error);
softmax values in bf16 (attention weights average the rounding away).
q-chunk projections and the output projection are emitted *interleaved* with
the attention inner loop so the PE fills the gaps while ScalarE (exp) is the
bottleneck.
"""

import numpy as np

B, N, E, H = 2, 2048, 1024, 16
D = E // H           # 64
NCORES = 8
HG = 4               # head groups
DH = E // HG         # 256 features per head-group
P = 128
NCH = N // 512       # 4 n-chunks of 512
ECH = E // P         # 8 contraction chunks
DCH = DH // P        # 2 feature chunks per shard
KT = N // P          # 16 key tiles
SCALE_COL = float(E ** 0.5)   # 32.0; row 64 of po = 32*rowsum

_CACHE = {}


def _build_program():
    import concourse.bacc as bacc
    import concourse.tile as tile
    from concourse import mybir

    F32 = mybir.dt.float32
    F32R = mybir.dt.float32r
    BF16 = mybir.dt.bfloat16
    EXP = mybir.ActivationFunctionType.Exp

    nc = bacc.Bacc(None, target_bir_lowering=False, debug=False)

    xqt = nc.declare_dram_parameter("xqt", [E, N], F32R, isOutput=False)
    xkt = nc.declare_dram_parameter("xkt", [E, N], F32R, isOutput=False)
    xvt = nc.declare_dram_parameter("xvt", [E, N], F32R, isOutput=False)
    wqt = nc.declare_dram_parameter("wqt", [E, DH], F32R, isOutput=False)
    wkt = nc.declare_dram_parameter("wkt", [E, DH], F32R, isOutput=False)
    wvt = nc.declare_dram_parameter("wvt", [E, DH], F32R, isOutput=False)
    wpt = nc.declare_dram_parameter("wpt", [DH, E], F32R, isOutput=False)
    bqp = nc.declare_dram_parameter("bq", [DCH, P, 1], F32, isOutput=False)
    bkp = nc.declare_dram_parameter("bk", [DCH, P, 1], F32, isOutput=False)
    bvp = nc.declare_dram_parameter("bv", [1, DH], F32R, isOutput=False)
    onesp = nc.declare_dram_parameter("ones", [1, P], F32R, isOutput=False)
    vonesp = nc.declare_dram_parameter("vones", [P, KT, HG, 1], BF16, isOutput=False)
    out = nc.declare_dram_parameter("out", [N, E], F32, isOutput=True)

    with tile.TileContext(nc) as tc:
        with (
            tc.tile_pool(name="singles", bufs=1) as singles,
            tc.tile_pool(name="xpool", bufs=3) as xpool,
            tc.tile_pool(name="spool", bufs=4) as spool,
            tc.tile_pool(name="npool", bufs=2) as npool,
            tc.tile_pool(name="opool", bufs=2) as opool,
            tc.tile_pool(name="pproj", bufs=1, space="PSUM") as pproj,
            tc.tile_pool(name="peps", bufs=2, space="PSUM") as peps,
            tc.tile_pool(name="ppo", bufs=2, space="PSUM") as ppo,
            tc.tile_pool(name="pbc", bufs=1, space="PSUM") as pbc,
        ):
            # ---- persistent weights / biases ----
            wq_sb = singles.tile([P, ECH, DH], F32R)
            wk_sb = singles.tile([P, ECH, DH], F32R)
            wv_sb = singles.tile([P, ECH, DH], F32R)
            wp_sb = singles.tile([P, DCH, E], F32R)
            nc.sync.dma_start(out=wq_sb, in_=wqt.rearrange("(c p) m -> p c m", p=P))
            nc.sync.dma_start(out=wk_sb, in_=wkt.rearrange("(c p) m -> p c m", p=P))
            nc.sync.dma_start(out=wv_sb, in_=wvt.rearrange("(c p) m -> p c m", p=P))
            nc.sync.dma_start(out=wp_sb, in_=wpt.rearrange("(c p) m -> p c m", p=P))
            bq_sb = singles.tile([P, DCH], F32)
            bk_sb = singles.tile([P, DCH], F32)
            for c in range(DCH):
                nc.sync.dma_start(out=bq_sb[:, c : c + 1], in_=bqp[c])
                nc.sync.dma_start(out=bk_sb[:, c : c + 1], in_=bkp[c])
            bv_sb = singles.tile([1, DH], F32R)
            nc.sync.dma_start(out=bv_sb, in_=bvp[:, :])
            ones1 = singles.tile([1, P], F32R)
            nc.sync.dma_start(out=ones1, in_=onesp[:, :])
            ones1_f = singles.tile([1, P], F32)
            nc.vector.memset(ones1_f, 1.0)

            qT_sb = singles.tile([P, DCH, N], F32R)
            kT_sb = singles.tile([P, DCH, N], F32R)
            oT_sb = singles.tile([P, DCH, N], F32R)
            v_sb = singles.tile([P, KT, HG, D + 1], BF16)
            nc.sync.dma_start(out=v_sb[:, :, :, D : D + 1], in_=vonesp[:, :, :, :])

            # ---- emit helpers ----
            def emit_kv_chunk(ni):
                ns = slice(ni * 512, (ni + 1) * 512)
                xk_c = xpool.tile([P, ECH, 512], F32R, tag="x", name=f"xk{ni}")
                nc.sync.dma_start(
                    out=xk_c, in_=xkt[:, ns].rearrange("(c p) n -> p c n", p=P)
                )
                for dc in range(DCH):
                    ps = pproj.tile([P, 512], F32, tag="proj", name=f"kps{ni}{dc}")
                    for ec in range(ECH):
                        nc.tensor.matmul(
                            ps,
                            wk_sb[:, ec, dc * P : (dc + 1) * P],
                            xk_c[:, ec, :],
                            start=(ec == 0),
                            stop=(ec == ECH - 1),
                        )
                    nc.vector.tensor_scalar_add(
                        kT_sb[:, dc, ns], ps, bk_sb[:, dc : dc + 1]
                    )
                xv_c = xpool.tile([P, ECH, 512], F32R, tag="x", name=f"xv{ni}")
                nc.sync.dma_start(
                    out=xv_c, in_=xvt[:, ns].rearrange("(c p) n -> p c n", p=P)
                )
                for k4 in range(4):
                    kt = ni * 4 + k4
                    vps = pproj.tile([P, DH], F32, tag="proj", name=f"vps{kt}")
                    nc.tensor.matmul(vps, ones1, bv_sb, start=True, stop=False)
                    for ec in range(ECH):
                        nc.tensor.matmul(
                            vps,
                            xv_c[:, ec, k4 * P : (k4 + 1) * P],
                            wv_sb[:, ec, :],
                            start=False,
                            stop=(ec == ECH - 1),
                        )
                    nc.vector.tensor_copy(
                        v_sb[:, kt, :, 0:D],
                        vps.rearrange("p (h d) -> p h d", h=HG),
                    )

            def q_proj_units(ni):
                """Deferred q-projection for chunk ni: DMA + one unit per dc."""
                ns = slice(ni * 512, (ni + 1) * 512)
                state = {}

                def dma_unit():
                    xq_c = xpool.tile([P, ECH, 512], F32R, tag="x", name=f"xq{ni}")
                    nc.sync.dma_start(
                        out=xq_c, in_=xqt[:, ns].rearrange("(c p) n -> p c n", p=P)
                    )
                    state["xq"] = xq_c

                def unit(dc):
                    xq_c = state["xq"]
                    ps = pproj.tile([P, 512], F32, tag="proj", name=f"qps{ni}{dc}")
                    for ec in range(ECH):
                        nc.tensor.matmul(
                            ps,
                            wq_sb[:, ec, dc * P : (dc + 1) * P],
                            xq_c[:, ec, :],
                            start=(ec == 0),
                            stop=(ec == ECH - 1),
                        )
                    nc.vector.tensor_scalar_add(
                        qT_sb[:, dc, ns], ps, bq_sb[:, dc : dc + 1]
                    )

                return [dma_unit] + [lambda dc=dc: unit(dc) for dc in range(DCH)]

            def outproj_units(qc):
                """Deferred output projection for q-chunk qc: 4 n-tile units."""

                def unit(nt):
                    n0 = qc * 512 + nt * P
                    osb = opool.tile([P, E], F32, tag="osb", name=f"osb{qc}{nt}")
                    for ecx in range(2):
                        ops = pproj.tile(
                            [P, 512], F32, tag="proj", name=f"ops{qc}{nt}{ecx}"
                        )
                        for dc in range(DCH):
                            nc.tensor.matmul(
                                ops,
                                oT_sb[:, dc, n0 : n0 + P],
                                wp_sb[:, dc, ecx * 512 : (ecx + 1) * 512],
                                start=(dc == 0),
                                stop=(dc == DCH - 1),
                            )
                        nc.vector.tensor_copy(
                            osb[:, ecx * 512 : (ecx + 1) * 512], ops
                        )
                    nc.sync.dma_start(out=out[n0 : n0 + P, :], in_=osb)

                return [lambda nt=nt: unit(nt) for nt in range(4)]

            def attn_groups(qc, pr, po, ktgs, slots=None, si0=0):
                qs = slice(qc * 512, (qc + 1) * 512)
                si = si0
                for ktg in ktgs:
                    eps = [
                        peps.tile([P, 1024], F32, tag="eps", name=f"eps{hp}")
                        for hp in range(2)
                    ]
                    for j in range(2):
                        kt = ktg * 2 + j
                        ks = slice(kt * P, (kt + 1) * P)
                        for hp in range(2):
                            rows = slice(hp * D, (hp + 1) * D)
                            nc.tensor.matmul(
                                eps[hp][:, j * 512 : (j + 1) * 512],
                                kT_sb[rows, pr, ks],
                                qT_sb[rows, pr, qs],
                                start=True,
                                stop=True,
                            )
                    sT = [
                        spool.tile([P, 1024], BF16, tag="sT", name=f"sT{hp}")
                        for hp in range(2)
                    ]
                    for hp in range(2):
                        nc.scalar.activation(sT[hp], eps[hp], EXP)
                    for j in range(2):
                        kt = ktg * 2 + j
                        for hp in range(2):
                            nc.tensor.matmul(
                                po[hp],
                                v_sb[:, kt, 2 * pr + hp, :],
                                sT[hp][:, j * 512 : (j + 1) * 512],
                                start=(kt == 0),
                                stop=(kt == KT - 1),
                            )
                    if slots is not None:
                        for u in slots[si]:
                            u()
                        si += 1

            def normalize(qc, pr, po):
                qs = slice(qc * 512, (qc + 1) * 512)
                for hp in range(2):
                    rinv = npool.tile([1, 512], F32, tag="rinv")
                    nc.vector.reciprocal(rinv, po[hp][D : D + 1, :])
                    o_tmp = npool.tile([D, 512], F32, tag="otmp")
                    nc.vector.tensor_copy(o_tmp, po[hp][0:D, :])
                    bc = pbc.tile([D, 512], F32, tag="bc")
                    nc.tensor.matmul(
                        bc, ones1_f[:, 0:D], rinv, start=True, stop=True
                    )
                    nc.vector.tensor_mul(
                        oT_sb[hp * D : (hp + 1) * D, pr, qs], o_tmp, bc
                    )

            def new_po():
                return [
                    ppo.tile([D + 1, 512], F32, tag="po", name=f"po{hp}")
                    for hp in range(2)
                ]

            # ---- emission: interleave qc=0 attention into the k/v loads so
            # ScalarE starts exp'ing as soon as the first k/v tiles land ----
            emit_kv_chunk(0)
            for u in q_proj_units(0):
                u()
            emit_kv_chunk(1)
            po0 = new_po()
            attn_groups(0, 0, po0, range(0, 4))        # ktiles 0-7 (kv 0,1)
            emit_kv_chunk(2)
            attn_groups(0, 0, po0, range(4, 6))        # ktiles 8-11 (kv 2)
            emit_kv_chunk(3)
            attn_groups(0, 0, po0, range(6, 8))        # ktiles 12-15 (kv 3)
            normalize(0, 0, po0)
            # q-chunk-1 projection interleaved into qc0/pr1 attention
            d0 = q_proj_units(1)
            slots0 = [[] for _ in range(KT // 2)]
            for i, u in enumerate(d0):
                slots0[(i * (KT // 2)) // len(d0)].append(u)
            po1 = new_po()
            attn_groups(0, 1, po1, range(0, 8), slots0, 0)
            normalize(0, 1, po1)

            # ---- remaining q-chunks with deferred work interleaved ----
            for qc in range(1, NCH):
                deferred = q_proj_units(qc + 1) if qc + 1 < NCH else []
                deferred += outproj_units(qc - 1)
                nslots = DCH * (KT // 2)
                slots = [[] for _ in range(nslots)]
                for i, u in enumerate(deferred):
                    slots[(i * nslots) // max(len(deferred), 1)].append(u)
                for pr in range(DCH):
                    po = new_po()
                    attn_groups(qc, pr, po, range(KT // 2), slots,
                                pr * (KT // 2))
                    normalize(qc, pr, po)
            # tail: output projection of the last q-chunk
            for u in outproj_units(NCH - 1):
                u()

    nc.compile()
    return nc


def _shard_inputs(queries, keys, values, Wq, bq, Wk, bk, Wv, bv):
    """Host-side shard/layout prep: feature-major activations, transposed
    weight shards.  Returns in_maps for the 8 cores."""
    import ml_dtypes

    f32 = np.float32
    xT = {}
    for name, x in (("xqt", queries), ("xkt", keys), ("xvt", values)):
        xT[name] = [np.ascontiguousarray(np.asarray(x[b], f32).T) for b in range(B)]
    maps = []
    for c in range(NCORES):
        b, hg = c // HG, c % HG
        rows = slice(hg * DH, (hg + 1) * DH)
        m = {
            "xqt": xT["xqt"][b],
            "xkt": xT["xkt"][b],
            "xvt": xT["xvt"][b],
            "wqt": np.ascontiguousarray(np.asarray(Wq, f32)[rows].T),
            "wkt": np.ascontiguousarray(np.asarray(Wk, f32)[rows].T),
            "wvt": np.ascontiguousarray(np.asarray(Wv, f32)[rows].T),
            "bq": np.asarray(bq, f32)[rows].reshape(DCH, P, 1).copy(),
            "bk": np.asarray(bk, f32)[rows].reshape(DCH, P, 1).copy(),
            "bv": np.asarray(bv, f32)[rows].reshape(1, DH).copy(),
            "ones": np.ones((1, P), f32),
            "vones": np.full((P, KT, HG, 1), SCALE_COL, ml_dtypes.bfloat16),
        }
        maps.append(m)
    return maps


def kernel(queries, keys, values, Wq, bq, Wk, bk, Wv, bv, Wp, bp):
    from concourse.bass_utils import run_bass_kernel_spmd

    if "nc" not in _CACHE:
        _CACHE["nc"] = _build_program()
    nc = _CACHE["nc"]

    in_maps = _shard_inputs(queries, keys, values, Wq, bq, Wk, bk, Wv, bv)
    Wp = np.asarray(Wp, np.float32)
    for c in range(NCORES):
        hg = c % HG
        rows = slice(hg * DH, (hg + 1) * DH)
        in_maps[c]["wpt"] = np.ascontiguousarray(Wp[:, rows].T)

    res = run_bass_kernel_spmd(nc, in_maps, list(range(NCORES)))

    out = np.zeros((B, N, E), np.float32)
    for c in range(NCORES):
        out[c // HG] += res.results[c]["out"]
    out += np.asarray(bp, np.float32)
    return out

